# revision 1
# baseline (speedup 1.0000x reference)
"""Trainium2 Bass kernel for GroupedQueryAttention with 1-bit quantized linears.

Sharding: 8 cores = 2 batches x 4 token-interleaved groups.
Core c handles batch b=c//4 and tokens t with t%4 == i (i=c%4), i.e. 512
query tokens per core.  Every core computes full K/V for its batch
(replicated), all 16 heads for its own 512 queries, and the full O
projection for those rows.  Host gathers by re-interleaving rows.

Program is identical across cores; all per-core variation is input data.
"""

import sys

sys.path.insert(0, "/opt/trn_rl_repo")

import numpy as np
import ml_dtypes

import concourse.bacc as bacc
import concourse.bass as bass
import concourse.mybir as mybir
import concourse.tile as tile
from concourse.masks import make_identity

F32 = mybir.dt.float32
F16 = mybir.dt.float16
BF16 = mybir.dt.bfloat16

B, T, D = 2, 2048, 2048
H, HK, HD = 16, 4, 128
G = 128
THETA = 1000000.0
NC = 8
TQ = T // 4          # 512 query tokens per core
QT = TQ // 128       # 4 query tiles
DT = D // 128        # 16 din tiles
KB = 512             # key block
NKB = T // KB        # 4 key blocks

ALPHA_K = 1.0 / G
ALPHA_Q = (HD ** -0.5) / G
SKIP = set()


def _bcast(ap_small, like_ap):
    """Broadcast ap_small (with size-1 dims) against like_ap shapes."""
    a, b = bass.broadcast_tensor_aps(like_ap, ap_small)
    return b


def build_program():
    nc = bacc.Bacc("TRN2", target_bir_lowering=False, debug=False, num_devices=NC)

    x = nc.dram_tensor("x", [T, D], F32, kind="ExternalInput").ap()
    xq = nc.dram_tensor("xq", [TQ, D], F32, kind="ExternalInput").ap()
    qw = nc.dram_tensor("qw", [H * HD, D], F32, kind="ExternalInput").ap()
    kw = nc.dram_tensor("kw", [HK * HD, D], F32, kind="ExternalInput").ap()
    vw = nc.dram_tensor("vw", [HK * HD, D], F32, kind="ExternalInput").ap()
    ow = nc.dram_tensor("ow", [D, H * HD], F32, kind="ExternalInput").ap()
    cosk = nc.dram_tensor("cosk", [HD, T], F16, kind="ExternalInput").ap()
    sinkr = nc.dram_tensor("sinkr", [HD, T], F16, kind="ExternalInput").ap()
    cosq = nc.dram_tensor("cosq", [HD, TQ], F16, kind="ExternalInput").ap()
    sinqr = nc.dram_tensor("sinqr", [HD, TQ], F16, kind="ExternalInput").ap()
    dmask = nc.dram_tensor("dmask", [128, 32], BF16, kind="ExternalInput").ap()
    out = nc.dram_tensor("out", [TQ, D], F32, kind="ExternalOutput").ap()

    with tile.TileContext(nc) as tc:
        build_tile_kernel(nc, tc, x, xq, qw, kw, vw, ow, cosk, sinkr, cosq,
                          sinqr, dmask, out)
    nc.compile()
    return nc


def build_tile_kernel(nc, tc, x, xq, qw, kw, vw, ow, cosk, sinkr, cosq, sinqr,
                      dmask, out):
    from contextlib import ExitStack

    ctx = ExitStack()
    with ctx:
        # ------- long-lived pools (open through the whole kernel) --------
        dram = ctx.enter_context(tc.tile_pool(name="dram", bufs=1, space="DRAM"))
        const = ctx.enter_context(tc.tile_pool(name="const", bufs=1))
        resid = ctx.enter_context(tc.tile_pool(name="resid", bufs=1))
        wpool = ctx.enter_context(tc.tile_pool(name="wstage", bufs=2))
        wqpool = ctx.enter_context(tc.tile_pool(name="wq", bufs=2))
        wtpool = ctx.enter_context(tc.tile_pool(name="wqT", bufs=2))
        rtmp = ctx.enter_context(tc.tile_pool(name="rtmp", bufs=2))

        xb = dram.tile([T, D], BF16)            # bf16 copy of x
        xqb = dram.tile([TQ, D], BF16)          # bf16 copy of xq
        wqd = dram.tile([D, H * HD], BF16)      # quantized o-weights (row major)

        dmask_sb = const.tile([128, 32], BF16)
        nc.sync.dma_start(dmask_sb, dmask)
        ones16k = const.tile([128, 1], BF16)
        nc.gpsimd.memset(ones16k, float(G * G))
        rq_dram = dram.tile([H, TQ], F32)   # per-head softmax reciprocal rows

        # residents alive to the end (64 KB/partition)
        QT_t = resid.tile([128, H, TQ], BF16)    # roped q^T  [d, h, t]
        KT_t = resid.tile([128, HK, T], BF16)    # roped k^T  [d, hk, t]
        V_t = resid.tile([128, T // 128, HK * HD], BF16)  # v [t, kv-dim]
        OT_t = resid.tile([128, H, TQ], BF16)    # attn out^T [dv, h, q]

        # ---------------- stage: cast x/xq to bf16 in DRAM ---------------
        nc.gpsimd.dma_start(xb[:], x)
        nc.gpsimd.dma_start(xqb[:], xq)

        # ---------------- weight quantization helper ---------------------
        def quant_tile(w_ap, row_tile, out_T=None, out_rowmajor=None):
            """Load 128 rows of w, 1-bit quantize -> bf16 (x G scale).

            If out_T given: write transposed [128(din), DT, 128(rows)] slice.
            If out_rowmajor given: DMA the row-major quantized tile there.
            """
            wst = wpool.tile([128, D], F32, tag="wst")
            nc.sync.dma_start(wst, w_ap[row_tile * 128:(row_tile + 1) * 128, :])
            ssum = wpool.tile([128, DT], F32, tag="ssum")
            nc.vector.tensor_reduce(
                ssum, wst.rearrange("p (g c) -> p g c", c=G),
                axis=mybir.AxisListType.X, op=mybir.AluOpType.add,
                apply_absolute_value=True)
            sgn = wqpool.tile([128, D], BF16, tag="sgn")
            nc.scalar.sign(sgn, wst)
            wqt = wqpool.tile([128, D], BF16, tag="wqt")
            sv = ssum.rearrange("p (g o) -> p g o", o=1)
            gv = sgn.rearrange("p (g c) -> p g c", c=G)
            nc.vector.tensor_tensor(
                wqt.rearrange("p (g c) -> p g c", c=G), gv, _bcast(sv, gv),
                op=mybir.AluOpType.mult)
            if out_T is not None:
                nc.sync.dma_start_transpose(out_T, wqt[:])
            if out_rowmajor is not None:
                nc.gpsimd.dma_start(out_rowmajor, wqt[:])

        # ---------------- rope helper ------------------------------------
        def rope_evac(ps, cos_sb, sinr_sb, col0, width, out_ap):
            """out = ps*cos + rot(ps)*sinr  (cast bf16)."""
            t1 = rtmp.tile([128, width], F32, tag="t1")
            t2 = rtmp.tile([128, width], F32, tag="t2")
            cs = cos_sb[:, col0:col0 + width]
            sr = sinr_sb[:, col0:col0 + width]
            nc.vector.tensor_tensor(t1, ps, cs, op=mybir.AluOpType.mult)
            nc.vector.tensor_tensor(t2[0:64, :], ps[64:128, :], sr[0:64, :],
                                    op=mybir.AluOpType.mult)
            nc.vector.tensor_tensor(t2[64:128, :], ps[0:64, :], sr[64:128, :],
                                    op=mybir.AluOpType.mult)
            nc.vector.tensor_add(out_ap, t1, t2)

        # ---------------- projections ------------------------------------
        with tc.tile_pool(name="xt", bufs=1) as pxt:
            XT = pxt.tile([128, DT, T], BF16)    # x^T (din-major), 64 KB/p
            for tc4 in range(4):
                nc.sync.dma_start_transpose(
                    XT[:, :, tc4 * 512:(tc4 + 1) * 512],
                    xb[tc4 * 512:(tc4 + 1) * 512, :])

            with tc.tile_pool(name="qph", bufs=1) as pq, \
                 tc.tile_pool(name="proj_ps", bufs=4, space="PSUM") as pps:
                XTq = pq.tile([128, DT, TQ], BF16)
                nc.sync.dma_start_transpose(XTq[:], xqb[:])
                cosq_sb = pq.tile([128, TQ], F16)
                sinqr_sb = pq.tile([128, TQ], F16)
                nc.sync.dma_start(cosq_sb, cosq)
                nc.sync.dma_start(sinqr_sb, sinqr)

                # Q: per head, out[d=128, TQ]
                for h in range(0 if "q" in SKIP else H):
                    wqT = wtpool.tile([128, DT, 128], BF16, tag="wqT")
                    quant_tile(qw, h, out_T=wqT[:])
                    ps = pps.tile([128, TQ], F32, tag="ps")
                    for dt in range(DT):
                        nc.tensor.matmul(ps, wqT[:, dt, :], XTq[:, dt, :],
                                         start=(dt == 0), stop=(dt == DT - 1))
                    rope_evac(ps, cosq_sb, sinqr_sb, 0, TQ, QT_t[:, h, :])

            with tc.tile_pool(name="kvph", bufs=1) as pkv, \
                 tc.tile_pool(name="proj_ps2", bufs=4, space="PSUM") as pps:
                cosk_sb = pkv.tile([128, T], F16)
                sinkr_sb = pkv.tile([128, T], F16)
                nc.sync.dma_start(cosk_sb, cosk)
                nc.sync.dma_start(sinkr_sb, sinkr)

                # K: per kv head, out[d=128, T] in 4 chunks
                for hk in range(0 if "k" in SKIP else HK):
                    wqT = wtpool.tile([128, DT, 128], BF16, tag="wqT")
                    quant_tile(kw, hk, out_T=wqT[:])
                    for tc4 in range(4):
                        ps = pps.tile([128, 512], F32, tag="psk")
                        for dt in range(DT):
                            nc.tensor.matmul(
                                ps, wqT[:, dt, :],
                                XT[:, dt, tc4 * 512:(tc4 + 1) * 512],
                                start=(dt == 0), stop=(dt == DT - 1))
                        rope_evac(ps, cosk_sb, sinkr_sb, tc4 * 512, 512,
                                  KT_t[:, hk, tc4 * 512:(tc4 + 1) * 512])

                # V: quantized weights transposed -> vq [din, DT, 512]
                vq = pkv.tile([128, DT, HK * HD], BF16)
                for rv in range(HK * HD // 128):
                    quant_tile(vw, rv, out_T=vq[:, :, rv * 128:(rv + 1) * 128])
                for tch in range(0 if "v" in SKIP else T // 128):
                    ps = pps.tile([128, HK * HD], F32, tag="psv")
                    for dt in range(DT):
                        nc.tensor.matmul(ps, XT[:, dt, tch * 128:(tch + 1) * 128],
                                         vq[:, dt, :],
                                         start=(dt == 0), stop=(dt == DT - 1))
                    nc.vector.tensor_copy(V_t[:, tch, :], ps)

        # o-weight quantization (overlaps attention)
        for ro in range(0 if "ow" in SKIP else D // 128):
            quant_tile(ow, ro, out_rowmajor=wqd[ro * 128:(ro + 1) * 128, :])

        # ---------------- attention (S^T layout) --------------------------
        # S^T[k, q] per (head, k-tile): lhsT = K^T tile, rhs = Q^T full.
        # Valid q range for k-tile kt is q >= 32*kt; the first 32 columns of
        # that range are the partial (diagonal) strip masked by dmask_sb.
        NKT = T // 128
        with tc.tile_pool(name="attn", bufs=2) as apool, \
             tc.tile_pool(name="st_ps", bufs=3, space="PSUM") as stp, \
             tc.tile_pool(name="sum_ps", bufs=2, space="PSUM") as sump, \
             tc.tile_pool(name="o_ps", bufs=2, space="PSUM") as op:
            for h in range(0 if "attn" in SKIP else H):
                hk = h // 4
                ps_o = op.tile([128, TQ], F32, tag="ps_o")
                ps_sum = sump.tile([1, TQ], F32, tag="ps_sum")
                pts = []
                for kt in range(NKT):
                    q0 = 32 * kt
                    ps_st = stp.tile([128, TQ], F32, tag="ps_st")
                    nc.tensor.matmul(ps_st[:, q0:], KT_t[:, hk, kt * 128:(kt + 1) * 128],
                                     QT_t[:, h, q0:], start=True, stop=True)
                    pt = apool.tile([128, TQ], BF16, tag="pt", bufs=NKT + 3)
                    nc.scalar.activation(pt[:, q0:], ps_st[:, q0:],
                                         mybir.ActivationFunctionType.Exp)
                    # diagonal strip mask (multiplicative 0/1)
                    nc.gpsimd.tensor_tensor(pt[:, q0:q0 + 32], pt[:, q0:q0 + 32],
                                            dmask_sb, op=mybir.AluOpType.mult)
                    pts.append(pt)
                for kt in range(NKT):
                    q0 = 32 * kt
                    pt = pts[kt]
                    nc.tensor.matmul(ps_sum[:, q0:], ones16k, pt[:, q0:],
                                     start=(kt == 0), stop=(kt == NKT - 1))
                    nc.tensor.matmul(ps_o[:, q0:], V_t[:, kt, hk * HD:(hk + 1) * HD],
                                     pt[:, q0:],
                                     start=(kt == 0), stop=(kt == NKT - 1))
                # reciprocal of sums, broadcast across partitions via DRAM
                rsum = apool.tile([1, TQ], F32, tag="rsum", bufs=2)
                nc.vector.reciprocal(rsum, ps_sum)
                nc.sync.dma_start(rq_dram[h:h + 1, :], rsum)
                RQ = apool.tile([128, TQ], F32, tag="RQ", bufs=2)
                src = rq_dram[h:h + 1, :]
                srcb, _ = bass.broadcast_tensor_aps(src, RQ[:])
                nc.sync.dma_start(RQ[:], srcb)
                nc.vector.tensor_tensor(OT_t[:, h, :], ps_o, RQ,
                                        op=mybir.AluOpType.mult)

        # ---------------- output projection ------------------------------
        with tc.tile_pool(name="owt", bufs=1) as pow_, \
             tc.tile_pool(name="oproj", bufs=3) as opool, \
             tc.tile_pool(name="op_ps", bufs=4, space="PSUM") as opp:
            owT = pow_.tile([128, H, D], BF16)   # [dH, ht, dout]
            for ht in range(H):
                nc.sync.dma_start_transpose(owT[:, ht, :],
                                            wqd[:, ht * 128:(ht + 1) * 128])
            for m in range(0 if "oproj" in SKIP else QT):
                osb = opool.tile([128, D], F32, tag="osb")
                for oc in range(4):
                    ps = opp.tile([128, 512], F32, tag="ps")
                    for ht in range(H):
                        nc.tensor.matmul(ps, OT_t[:, ht, m * 128:(m + 1) * 128],
                                         owT[:, ht, oc * 512:(oc + 1) * 512],
                                         start=(ht == 0), stop=(ht == H - 1))
                    nc.vector.tensor_copy(osb[:, oc * 512:(oc + 1) * 512], ps)
                nc.sync.dma_start(out[m * 128:(m + 1) * 128, :], osb)


# ---------------------------------------------------------------------------
# host side
# ---------------------------------------------------------------------------
_CACHE = {}


def _tables():
    inv = 1.0 / (THETA ** (np.arange(0, HD, 2, dtype=np.float64) / HD))
    t = np.arange(T, dtype=np.float64)
    fr = np.outer(t, inv)                      # [T, 64]
    emb = np.concatenate([fr, fr], axis=1)     # [T, 128]
    cosT = np.cos(emb).T                       # [128, T] float64
    sinT = np.sin(emb).T
    sinr = np.empty_like(sinT)
    sinr[0:64] = -sinT[0:64]
    sinr[64:128] = sinT[64:128]
    return cosT, sinT, sinr


def make_in_maps(hidden, q_w, k_w, v_w, o_w):
    cosT, sinT, sinr = _tables()
    f16 = np.float16
    in_maps = []
    for c in range(NC):
        b, i = c // 4, c % 4
        xb_ = np.ascontiguousarray(hidden[b])
        xq_ = np.ascontiguousarray(hidden[b][i::4, :])
        cq = np.ascontiguousarray(cosT[:, i::4] * ALPHA_Q).astype(f16)
        sq = np.ascontiguousarray(sinr[:, i::4] * ALPHA_Q).astype(f16)
        # dmask[r, c] = 1 iff key-local r <= 4c + i (diagonal 128x32 strip)
        r = np.arange(128)[:, None]
        cc = np.arange(32)[None, :]
        dm = (r <= 4 * cc + i).astype(ml_dtypes.bfloat16)
        in_maps.append({
            "x": xb_, "xq": xq_, "qw": q_w, "kw": k_w, "vw": v_w, "ow": o_w,
            "cosk": np.ascontiguousarray(cosT * ALPHA_K).astype(f16),
            "sinkr": np.ascontiguousarray(sinr * ALPHA_K).astype(f16),
            "cosq": cq, "sinqr": sq, "dmask": dm,
        })
    return in_maps


def kernel(hidden, q_w, k_w, v_w, o_w):
    hidden = np.asarray(hidden, dtype=np.float32)
    q_w = np.ascontiguousarray(np.asarray(q_w, dtype=np.float32))
    k_w = np.ascontiguousarray(np.asarray(k_w, dtype=np.float32))
    v_w = np.ascontiguousarray(np.asarray(v_w, dtype=np.float32))
    o_w = np.ascontiguousarray(np.asarray(o_w, dtype=np.float32))

    if "nc" not in _CACHE:
        _CACHE["nc"] = build_program()
    nc = _CACHE["nc"]

    in_maps = make_in_maps(hidden, q_w, k_w, v_w, o_w)
    from concourse.bass_utils import run_bass_kernel_spmd
    res = run_bass_kernel_spmd(nc, in_maps, core_ids=list(range(NC)))
    out = np.empty((B, T, D), dtype=np.float32)
    for c in range(NC):
        b, i = c // 4, c % 4
        out[b, i::4, :] = res.results[c]["out"]
    return out


if __name__ == "__main__":
    print("building program...")
    nc = build_program()
    print("BUILD OK")

